# revision 1
# baseline (speedup 1.0000x reference)
"""DRQConv2d (dual-region quantized conv) Trainium2 kernel.

Reference semantics (see problem statement):
  mask  = upsample8(avgpool8(x) >= 0.05)             per (b, c)
  xh    = where(mask, x, 1e-5);  xl = where(mask, 1e-5, x)
  qh    = clip(round(xh/sh), 0, 255) * sh            (uint8 fake-quant)
  ql    = clip(round(xl/sl), 0, 15) * sl             (uint4 fake-quant)
  qwh   = per-oc quant of w_high to +-127,  qwl = per-oc quant of w_low to +-7
  y     = conv3x3(qh, qwh) + conv3x3(ql, qwl)        (pad 1)

Key facts exploited here:
  * 1e-5 quantizes to exactly 0 on both paths, so the masked fill is just a
    multiply by the {0,1} mask after rounding.
  * The quantized activations and weights are exact small integers
    (<=255 / <=127), which bf16 represents exactly; PSUM accumulates fp32.
    So bf16 matmuls reproduce the fp32 reference up to accumulation order.
  * conv3x3 = 9 shift-offset matmuls (K=C_in=128) accumulated in PSUM over a
    zero-padded 58x58 activation layout.

Sharding: data-parallel over batch. 32 images -> 4 per core on 8 cores,
weights replicated; outputs concatenated on host. No collectives.
"""

import numpy as np

P = 128            # channels (both in and out) == partitions
B_TOTAL = 32
N_CORES = 8
BPC = B_TOTAL // N_CORES   # images per core
H = W = 56
HP = WP = H + 2    # zero-padded layout
NPIX = H * W       # 3136
NPAD = HP * WP     # 3364
NTAPS = 9
ROWS_PER_CHUNK = 8
NCHUNK = H // ROWS_PER_CHUNK          # 7
NFREE = ROWS_PER_CHUNK * W            # 448 columns per matmul
MAGIC = float(np.float32(1.5 * 2 ** 23))   # fp32 round-to-nearest magic
POOL_K = 8
THRESH = 0.05


def build_program(nc, tc, aps, inv_sh, inv_sl, c_svh, c_svl, bpc=BPC):
    """Emit the whole per-core program inside an open TileContext.

    aps: dict with DRAM APs: x [bpc,P,NPIX], w_high [P,1152], w_low [P,1152],
         y [bpc,P,NPIX].
    inv_sh/inv_sl: 1/act_scale (host floats, baked as immediates).
    c_svh/c_svl: act_scale / (2^(b-1)-1) -- multiplied by per-oc |w|max to give
         the combined output scale.
    """
    import concourse.mybir as mybir
    from concourse.alu_op_type import AluOpType as op
    from concourse.masks import make_identity

    f32 = mybir.dt.float32
    bf16 = mybir.dt.bfloat16
    X = mybir.AxisListType.X

    x_d, wh_d, wl_d, y_d = aps["x"], aps["w_high"], aps["w_low"], aps["y"]

    sum_thresh = float(np.float32(THRESH) * POOL_K * POOL_K)  # exact pow2 scale

    with (
        tc.tile_pool(name="consts", bufs=1) as consts,
        tc.tile_pool(name="wtmp", bufs=2) as wtmp_pool,
        tc.tile_pool(name="tp_psum", bufs=1, space="PSUM") as tp_psum,
        tc.tile_pool(name="acts", bufs=2) as acts,
        tc.tile_pool(name="masks", bufs=2) as maskp,
        tc.tile_pool(name="qtiles", bufs=4) as qtiles,
        tc.tile_pool(name="outs", bufs=2) as outs_pool,
        tc.tile_pool(name="conv_psum", bufs=7, space="PSUM") as conv_psum,
    ):
        identity = consts.tile([P, P], f32)
        make_identity(nc, identity[:])

        qwt = {}   # conv -> bf16 [P(ic), 9*P(oc)] transposed integer weights
        sv = {}    # conv -> f32 [P(oc), 1] combined output scale

        def weight_prep(conv, w_dram, nw, c_sv, scale_by_ratio):
            """Quantize weights per-oc; 'l' weights additionally pre-scaled by
            sv_l/sv_h so the low conv can accumulate into the high conv's PSUM
            (single final scale by sv_h)."""
            wnat = wtmp_pool.tile([P, P * NTAPS], f32, tag="wnat")
            nc.sync.dma_start(out=wnat[:], in_=w_dram)
            absmax = consts.tile([P, 1], f32, tag=f"absmax_{conv}")
            nc.vector.tensor_reduce(
                absmax[:], wnat[:], axis=X, op=op.max, apply_absolute_value=True
            )
            sv_t = consts.tile([P, 1], f32, tag=f"sv_{conv}")
            nc.vector.tensor_scalar_mul(sv_t[:], absmax[:], c_sv)
            sv[conv] = sv_t
            rcp = consts.tile([P, 1], f32, tag=f"rcp_{conv}")
            nc.vector.reciprocal(rcp[:], absmax[:])
            rs = consts.tile([P, 1], f32, tag=f"rs_{conv}")
            nc.vector.tensor_scalar_mul(rs[:], rcp[:], nw)

            # integer-quantize in natural [oc, ic*9] layout (per-partition scalar)
            wq = wtmp_pool.tile([P, P * NTAPS], f32, tag="wq")
            nc.vector.tensor_scalar(
                wq[:], wnat[:], rs[:, 0:1], MAGIC, op0=op.mult, op1=op.add
            )
            nc.vector.tensor_scalar(
                wq[:], wq[:], MAGIC, nw, op0=op.subtract, op1=op.min
            )
            nc.vector.tensor_scalar_max(wq[:], wq[:], -nw)
            if scale_by_ratio:
                rcp_svh = consts.tile([P, 1], f32)
                nc.vector.reciprocal(rcp_svh[:], sv["h"][:, 0:1])
                ratio = consts.tile([P, 1], f32)
                nc.vector.tensor_tensor(
                    ratio[:], sv_t[:], rcp_svh[:], op=op.mult
                )
                nc.vector.tensor_scalar_mul(wq[:], wq[:], ratio[:, 0:1])

            # transpose each 3x3 tap: [oc, ic] -> [ic, oc], cast to bf16
            qwt_t = consts.tile([P, NTAPS * P], bf16, tag=f"qwt_{conv}")
            wq_v = wq[:].rearrange("p (i t) -> p t i", t=NTAPS)
            for base in range(0, NTAPS, 4):
                n = min(4, NTAPS - base)
                tp = tp_psum.tile([P, 4 * P], f32, tag="tp")
                for j in range(n):
                    nc.tensor.transpose(
                        tp[:, j * P:(j + 1) * P],
                        wq_v[:, base + j, :], identity[:],
                    )
                nc.vector.tensor_copy(
                    out=qwt_t[:, base * P:(base + n) * P], in_=tp[:, :n * P]
                )
            qwt[conv] = qwt_t

        def mask_prep(b, xt):
            """Block sums -> threshold -> full-res {0,1} masks [P, NPIX].

            The w-blocksum reduce writes its output TRANSPOSED to (wb, h)
            order so the h-blocksum is a single contiguous-group reduce;
            the threshold result is fixed back to (hb, wb) with a tiny copy.
            (gpsimd deliberately unused here: it contends with DVE for the
            shared SBUF port.)"""
            r1 = acts.tile([P, H * NCHUNK], f32, tag="r1")   # [P, 392] (wb, h)
            nc.vector.reduce_sum(
                r1[:].rearrange("p (w h) -> p h w", w=NCHUNK),
                xt[:].rearrange("p (r c) -> p r c", c=POOL_K),
                axis=X,
            )
            r2 = acts.tile([P, NCHUNK * NCHUNK], f32, tag="r2")  # [P,49] (wb,hb)
            nc.vector.reduce_sum(
                r2[:], r1[:].rearrange("p (g c) -> p g c", c=POOL_K), axis=X
            )
            mt = acts.tile([P, NCHUNK * NCHUNK], f32, tag="mt")
            nc.vector.tensor_scalar(
                mt[:], r2[:], sum_thresh, None, op0=op.is_ge
            )
            # fix-up to (hb, wb) order with a tiny transposing copy
            m = acts.tile([P, NCHUNK * NCHUNK], f32, tag="m")
            nc.vector.tensor_copy(
                out=m[:], in_=mt[:].rearrange("p (w h) -> p h w", w=NCHUNK)
            )
            # expand to full res: [P,49] -> [P,392] (DVE) -> [P,3136] (ACT)
            mr = acts.tile([P, NCHUNK * W], f32, tag="mr")
            nc.vector.tensor_copy(
                out=mr[:].rearrange("p (r c) -> p r c", c=POOL_K),
                in_=m[:].unsqueeze(2).broadcast_to((P, NCHUNK * NCHUNK, POOL_K)),
            )
            mexp_h = maskp.tile([P, NPIX], f32, tag="mexp_h")
            mh3 = mexp_h[:].rearrange("p (r c) -> p r c", r=H)
            for hb in range(NCHUNK):
                nc.vector.tensor_copy(
                    out=mh3[:, hb * POOL_K:(hb + 1) * POOL_K, :],
                    in_=mr[:, hb * W:(hb + 1) * W]
                    .unsqueeze(1).broadcast_to((P, POOL_K, W)),
                )
            mexp_l = maskp.tile([P, NPIX], f32, tag="mexp_l")
            nc.vector.tensor_scalar(
                mexp_l[:], mexp_h[:], -1.0, 1.0, op0=op.mult, op1=op.add
            )
            return {"h": mexp_h, "l": mexp_l}

        def quant_act(b, xt, mexp, conv, inv_s, qmax):
            """relu/scale (ACT) -> min+round (DVE) -> mask-mult into padded
            bf16 tile."""
            r = acts.tile([P, NPIX], f32, tag="r")
            nc.scalar.activation(
                r[:], xt[:], mybir.ActivationFunctionType.Relu, scale=inv_s
            )
            t = r
            nc.vector.tensor_scalar(
                t[:], r[:], qmax, MAGIC, op0=op.min, op1=op.add
            )
            q = qtiles.tile([P, NPAD], bf16, tag="q")
            q2 = q[:].rearrange("p (r c) -> p r c", r=HP)
            # zero borders: rows 0,57 and cols 0,57 (gpsimd; keeps DVE free)
            nc.gpsimd.memset(q2[:, 0:HP:HP - 1, :], 0.0)
            nc.gpsimd.memset(q2[:, 1:HP - 1, 0:WP:WP - 1], 0.0)
            nc.vector.scalar_tensor_tensor(
                out=q2[:, 1:H + 1, 1:W + 1],
                in0=t[:].rearrange("p (r c) -> p r c", r=H),
                scalar=MAGIC,
                in1=mexp[conv][:].rearrange("p (r c) -> p r c", r=H),
                op0=op.subtract, op1=op.mult,
            )
            return q

        def conv_pass(qa_tile, conv, pss, first):
            """One full conv pass (9 taps x 7 chunks) accumulating into the
            7 live PSUM banks. Low weights are pre-scaled by sv_l/sv_h so both
            passes share banks and a single evacuation."""
            q2 = qa_tile[:].rearrange("p (r c) -> p r c", r=HP)
            for c in range(NCHUNK):
                r0 = c * ROWS_PER_CHUNK
                for tap in range(NTAPS):
                    kh, kw = divmod(tap, 3)
                    rhs = q2[:, r0 + kh:r0 + kh + ROWS_PER_CHUNK, kw:kw + W]
                    nc.tensor.matmul(
                        pss[c][:], qwt[conv][:, tap * P:(tap + 1) * P], rhs,
                        start=(first and tap == 0),
                        stop=(not first and tap == NTAPS - 1),
                    )

        def conv_image(b, qa):
            acc = outs_pool.tile([P, NPIX], f32, tag="acc")
            pss = [conv_psum.tile([P, NFREE], f32, tag="ps", name=f"ps{b}_{c}")
                   for c in range(NCHUNK)]
            conv_pass(qa["h"], "h", pss, True)
            conv_pass(qa["l"], "l", pss, False)
            for c in range(NCHUNK):
                r0 = c * ROWS_PER_CHUNK
                seg = acc[:, r0 * W:(r0 + ROWS_PER_CHUNK) * W]
                nc.scalar.mul(seg, pss[c][:], sv["h"][:, 0:1])
                nc.sync.dma_start(
                    out=y_d[b][:, r0 * W:(r0 + ROWS_PER_CHUNK) * W], in_=seg
                )

        # ---------------- schedule ----------------
        xts = {}
        xts[0] = acts.tile([P, NPIX], f32, tag="xt", name="xt0")
        nc.sync.dma_start(out=xts[0][:], in_=x_d[0])

        weight_prep("h", wh_d, 127.0, c_svh, False)
        weight_prep("l", wl_d, 7.0, c_svl, True)

        # PE warm-up: HAM un-throttles after ~3.4us of sustained activity;
        # burn idle startup time on dummy matmuls so the real work runs at
        # 2.4 GHz from the first transpose.
        warm_ps = tp_psum.tile([P, 4 * P], f32, tag="tp")
        for i in range(28):
            nc.tensor.matmul(
                warm_ps[:, 0:P], identity[:], identity[:],
                start=(i == 0), stop=(i == 27),
            )

        for b in range(bpc):
            if b not in xts:
                xts[b] = acts.tile([P, NPIX], f32, tag="xt", name=f"xt{b}")
                nc.sync.dma_start(out=xts[b][:], in_=x_d[b])
            xt = xts[b]
            mexp = mask_prep(b, xt)
            qa = {
                "h": quant_act(b, xt, mexp, "h", inv_sh, 255.0),
                "l": quant_act(b, xt, mexp, "l", inv_sl, 15.0),
            }
            conv_image(b, qa)


def make_bass(inv_sh, inv_sl, c_svh, c_svl, bpc=BPC):
    import concourse.bacc as bacc
    import concourse.mybir as mybir
    from concourse.tile import TileContext

    f32 = mybir.dt.float32
    nc = bacc.Bacc("TRN2", debug=False)
    x = nc.dram_tensor("x", [bpc, P, NPIX], f32, kind="ExternalInput")
    wh = nc.dram_tensor("w_high", [P, P * NTAPS], f32, kind="ExternalInput")
    wl = nc.dram_tensor("w_low", [P, P * NTAPS], f32, kind="ExternalInput")
    y = nc.dram_tensor("y", [bpc, P, NPIX], f32, kind="ExternalOutput")
    aps = {"x": x.ap(), "w_high": wh.ap(), "w_low": wl.ap(), "y": y.ap()}
    with TileContext(nc) as tc:
        build_program(nc, tc, aps, inv_sh, inv_sl, c_svh, c_svl, bpc=bpc)
    nc.compile()
    return nc


def _scale_consts(act_scale_high, act_scale_low):
    sh = float(np.float32(act_scale_high))
    sl = float(np.float32(act_scale_low))
    inv_sh = float(np.float32(1.0 / np.float64(sh)))
    inv_sl = float(np.float32(1.0 / np.float64(sl)))
    c_svh = float(np.float32(np.float64(sh) / 127.0))
    c_svl = float(np.float32(np.float64(sl) / 7.0))
    return inv_sh, inv_sl, c_svh, c_svl


def _run(x, w_high, w_low, act_scale_high, act_scale_low, trace=False, **kw):
    from concourse import bass_utils

    x = np.ascontiguousarray(np.asarray(x, dtype=np.float32))
    w_high = np.ascontiguousarray(np.asarray(w_high, dtype=np.float32))
    w_low = np.ascontiguousarray(np.asarray(w_low, dtype=np.float32))

    inv_sh, inv_sl, c_svh, c_svl = _scale_consts(act_scale_high, act_scale_low)
    nc = make_bass(inv_sh, inv_sl, c_svh, c_svl)

    wh_flat = w_high.reshape(P, P * NTAPS)
    wl_flat = w_low.reshape(P, P * NTAPS)
    in_maps = []
    for core in range(N_CORES):
        xs = x[core * BPC:(core + 1) * BPC].reshape(BPC, P, NPIX)
        in_maps.append(
            {
                "x": np.ascontiguousarray(xs),
                "w_high": wh_flat,
                "w_low": wl_flat,
            }
        )
    res = bass_utils.run_bass_kernel_spmd(
        nc, in_maps, core_ids=list(range(N_CORES)), trace=trace, **kw
    )
    y = np.concatenate([r["y"].reshape(BPC, P, H, W) for r in res.results], axis=0)
    return y, res


def kernel(x, w_high, w_low, act_scale_high, act_scale_low):
    y, _ = _run(x, w_high, w_low, act_scale_high, act_scale_low)
    return y



# revision 6
# speedup vs baseline: 1.1656x; 1.1656x over previous
"""DRQConv2d (dual-region quantized conv) Trainium2 kernel — v2.

Reference semantics:
  mask  = upsample8(avgpool8(x) >= 0.05)             per (b, c)
  xh    = where(mask, x, 1e-5);  xl = where(mask, 1e-5, x)
  qh    = clip(round(xh/sh), 0, 255) * sh            (uint8 fake-quant)
  ql    = clip(round(xl/sl), 0, 15) * sl             (uint4 fake-quant)
  qwh   = per-oc quant of w_high to +-127,  qwl = per-oc quant of w_low to +-7
  y     = conv3x3(qh, qwh) + conv3x3(ql, qwl)        (pad 1)

Key ideas vs the v1 baseline (151us):
  * Low conv runs in fp8 e4m3 with MatmulPerfMode.DoubleRow: quantized low
    activations (ints 0..15) and weights (ints +-7) are exactly representable
    in e4m3, and DoubleRow packs TWO 3x3 taps (2x128 contraction rows) into
    one PE instruction -> 9 taps in 5 matmuls instead of 9.  The per-oc scale
    ratio sv_h/sv_l is folded into the HIGH (bf16) weights instead of the low
    ones so the low weights stay exact integers; both convs share PSUM banks
    and one final evacuation scale by sv_l (adds ~0.1% error, gate is 2%).
  * The region mask is applied as a CLAMP BOUND instead of a multiply:
    v = x/s + MAGIC (ACT, scale+round fused); u = min(v, MAGIC + qmax*mask)
    (GpSimd, mask expanded only to [P, 7*56] block-row resolution and
    broadcast-viewed); q = max(u, MAGIC) - MAGIC (DVE tensor_scalar, 2x mode).
    Masked-out pixels clamp to exactly MAGIC -> quantize to 0, which matches
    the reference (1e-5 rounds to 0).  This removes the full-res mask
    expansion and the 1x-mode scalar_tensor_tensor of v1.
  * PSUM is laid out as two bank-aligned supertiles (chunks at 512-elem
    stride) so each image needs only 2 strided ACT evacuations + 1 DMA out.

Sharding: data-parallel over batch. 32 images -> 4 per core on 8 cores,
weights replicated; outputs concatenated on host. No collectives.
"""

import numpy as np

P = 128            # channels (both in and out) == partitions
B_TOTAL = 32
N_CORES = 8
BPC = B_TOTAL // N_CORES   # images per core
H = W = 56
HP = WP = H + 2    # zero-padded layout
NPIX = H * W       # 3136
NPAD = HP * WP     # 3364
NTAPS = 9
ROWS = 8
NCHUNK = H // ROWS                    # 7
NFREE = ROWS * W                      # 448 columns per matmul
BANK = 512                            # PSUM bank stride in f32 elements
MAGIC = float(np.float32(1.5 * 2 ** 23))   # fp32 round-to-nearest magic
POOL_K = 8
THRESH = 0.05


def build_program(nc, tc, aps, inv_sh, inv_sl, c_svh, c_svl, bpc=BPC):
    import bass_rust as _br
    import concourse.mybir as mybir
    from concourse.alu_op_type import AluOpType as op
    from concourse.masks import make_identity

    f32 = mybir.dt.float32
    bf16 = mybir.dt.bfloat16
    fp8 = mybir.dt.float8e4
    X = mybir.AxisListType.X
    DR = mybir.MatmulPerfMode.DoubleRow
    IDENT = mybir.ActivationFunctionType.Identity

    x_d, wh_d, wl_d, y_d = aps["x"], aps["w_high"], aps["w_low"], aps["y"]
    sum_thresh = float(np.float32(THRESH) * POOL_K * POOL_K)  # exact pow2 scale

    with (
        tc.tile_pool(name="consts", bufs=1) as consts,
        tc.tile_pool(name="wtmp", bufs=1) as wtmp,
        tc.tile_pool(name="psum", bufs=1, space="PSUM") as psum_pool,
        tc.tile_pool(name="acts", bufs=2) as acts,
        tc.tile_pool(name="qtiles", bufs=2) as qtiles,
        tc.tile_pool(name="outs", bufs=2) as outs_pool,
    ):
        identity = consts.tile([P, P], f32)
        make_identity(nc, identity[:])
        magic_ap = consts.tile([P, 1], f32, tag="magic", name="magic")
        nc.vector.memset(magic_ap[:], MAGIC)

        # PSUM: chunks 0-3 in psA banks 0-3, chunks 4-6 in psB banks 0-2;
        # psB bank 3 (the 8th PSUM bank) doubles as transpose/warmup scratch
        # (only touched before the first conv matmul).
        psA = psum_pool.tile([P, 4 * BANK], f32, tag="psA")
        psB = psum_pool.tile([P, 4 * BANK], f32, tag="psB")
        tp = psB[:, 3 * BANK:4 * BANK]

        # ---------------- weight prep ----------------
        wnat_h = wtmp.tile([P, P * NTAPS], f32, tag="wnat_h")
        wnat_l = wtmp.tile([P, P * NTAPS], f32, tag="wnat_l")
        nc.sync.dma_start(out=wnat_h[:], in_=wh_d)
        nc.sync.dma_start(out=wnat_l[:], in_=wl_d)

        def pp(tag):
            return consts.tile([P, 1], f32, tag=tag, name=tag)

        am_h, am_l = pp("am_h"), pp("am_l")
        nc.vector.tensor_reduce(
            am_h[:], wnat_h[:], axis=X, op=op.max, apply_absolute_value=True)
        nc.vector.tensor_reduce(
            am_l[:], wnat_l[:], axis=X, op=op.max, apply_absolute_value=True)
        sv_l, sv_h = pp("sv_l"), pp("sv_h")
        nc.vector.tensor_scalar_mul(sv_l[:], am_l[:], c_svl)
        nc.vector.tensor_scalar_mul(sv_h[:], am_h[:], c_svh)
        rcp_svl, ratio = pp("rcp_svl"), pp("ratio")
        nc.vector.reciprocal(rcp_svl[:], sv_l[:])
        nc.vector.tensor_tensor(ratio[:], sv_h[:], rcp_svl[:], op=op.mult)
        rcp_h, rs_h = pp("rcp_h"), pp("rs_h")
        nc.vector.reciprocal(rcp_h[:], am_h[:])
        nc.vector.tensor_scalar_mul(rs_h[:], rcp_h[:], 127.0)
        rcp_l, rs_l = pp("rcp_l"), pp("rs_l")
        nc.vector.reciprocal(rcp_l[:], am_l[:])
        nc.vector.tensor_scalar_mul(rs_l[:], rcp_l[:], 7.0)

        # integer-quantize in natural [oc, ic*9] layout, in place
        nc.vector.tensor_scalar(
            wnat_h[:], wnat_h[:], rs_h[:, 0:1], MAGIC, op0=op.mult, op1=op.add)
        nc.vector.tensor_scalar(
            wnat_h[:], wnat_h[:], MAGIC, 127.0, op0=op.subtract, op1=op.min)
        # fold sv_h/sv_l into the high weights so both convs share one
        # output scale (sv_l); low weights stay exact fp8 integers
        nc.vector.tensor_scalar(
            wnat_h[:], wnat_h[:], -127.0, ratio[:, 0:1], op0=op.max, op1=op.mult)
        nc.vector.tensor_scalar(
            wnat_l[:], wnat_l[:], rs_l[:, 0:1], MAGIC, op0=op.mult, op1=op.add)
        nc.vector.tensor_scalar(
            wnat_l[:], wnat_l[:], MAGIC, 7.0, op0=op.subtract, op1=op.min)
        nc.vector.tensor_scalar_max(wnat_l[:], wnat_l[:], -7.0)

        # transpose each tap [oc, ic] -> [ic, oc]; cast to bf16 / fp8e4
        qwt_h = consts.tile([P, NTAPS * P], bf16, tag="qwt_h")
        qwt_l = consts.tile([P, NTAPS * P], fp8, tag="qwt_l")
        for wq, qwt in ((wnat_h, qwt_h), (wnat_l, qwt_l)):
            wv = wq[:].rearrange("p (i t) -> p t i", t=NTAPS)
            for base in range(0, NTAPS, 4):
                n = min(4, NTAPS - base)
                for j in range(n):
                    nc.tensor.transpose(
                        tp[:, j * P:(j + 1) * P], wv[:, base + j, :], identity[:])
                nc.vector.tensor_copy(
                    out=qwt[:, base * P:(base + n) * P], in_=tp[:, :n * P])

        # PE warm-up: HAM un-throttles after ~3.4us of sustained activity
        for i in range(28):
            nc.tensor.matmul(
                tp[:, 0:P], identity[:], identity[:],
                start=(i == 0), stop=(i == 27))

        # ---------------- per-image pipeline ----------------
        xts = {}

        def issue_x(b):
            if b < bpc and b not in xts:
                xts[b] = acts.tile([P, NPIX], f32, tag="xt", name=f"xt{b}")
                nc.sync.dma_start(out=xts[b][:], in_=x_d[b])

        issue_x(0)

        for b in range(bpc):
            issue_x(b)
            issue_x(b + 1)
            xt = xts[b]

            # --- mask -> clamp bounds at block-row resolution ---
            # (w-blocksum written transposed so h-blocksum is contiguous)
            r1 = acts.tile([P, H * NCHUNK], f32, tag="r1")
            nc.vector.reduce_sum(
                r1[:].rearrange("p (w h) -> p h w", w=NCHUNK),
                xt[:].rearrange("p (r c) -> p r c", c=POOL_K), axis=X)
            r2 = acts.tile([P, NCHUNK * NCHUNK], f32, tag="r2")
            nc.vector.reduce_sum(
                r2[:], r1[:].rearrange("p (g c) -> p g c", c=POOL_K), axis=X)
            mt = acts.tile([P, NCHUNK * NCHUNK], f32, tag="mt")
            nc.vector.tensor_scalar(mt[:], r2[:], sum_thresh, None, op0=op.is_ge)
            m = acts.tile([P, NCHUNK * NCHUNK], f32, tag="m")
            nc.vector.tensor_copy(
                out=m[:], in_=mt[:].rearrange("p (w h) -> p h w", w=NCHUNK))
            # bound_h = MAGIC + 255*m ; bound_l = MAGIC + 15*(1-m)
            bh_blk = acts.tile([P, NCHUNK * NCHUNK], f32, tag="bh_blk")
            nc.vector.tensor_scalar(
                bh_blk[:], m[:], 255.0, MAGIC, op0=op.mult, op1=op.add)
            bl_blk = acts.tile([P, NCHUNK * NCHUNK], f32, tag="bl_blk")
            nc.vector.tensor_scalar(
                bl_blk[:], m[:], -15.0, MAGIC + 15.0, op0=op.mult, op1=op.add)
            bh = acts.tile([P, NCHUNK * W], f32, tag="bh")
            nc.vector.tensor_copy(
                out=bh[:].rearrange("p (g c) -> p g c", c=POOL_K),
                in_=bh_blk[:].unsqueeze(2).broadcast_to(
                    (P, NCHUNK * NCHUNK, POOL_K)))
            bl = acts.tile([P, NCHUNK * W], f32, tag="bl")
            nc.vector.tensor_copy(
                out=bl[:].rearrange("p (g c) -> p g c", c=POOL_K),
                in_=bl_blk[:].unsqueeze(2).broadcast_to(
                    (P, NCHUNK * NCHUNK, POOL_K)))

            # --- quantize: ACT scale+round, GpSimd mask-clamp, DVE finish ---
            def quant(conv, inv_s, bnd, qdt):
                v = acts.tile([P, NPIX], f32, tag=f"v{conv}")
                nc.scalar.activation(
                    v[:], xt[:], IDENT, bias=magic_ap[:, 0:1], scale=inv_s)
                v4 = v[:].rearrange("p (hb r c) -> p hb r c", hb=NCHUNK, r=ROWS)
                bnd4 = (bnd[:].rearrange("p (hb c) -> p hb c", hb=NCHUNK)
                        .unsqueeze(2).broadcast_to((P, NCHUNK, ROWS, W)))
                nc.vector.tensor_tensor(v4, v4, bnd4, op=op.min)
                q = qtiles.tile([P, NPAD], qdt, tag=f"q{conv}")
                q3 = q[:].rearrange("p (r c) -> p r c", r=HP)
                nc.gpsimd.memset(q3[:, 0:HP:HP - 1, :], 0.0)
                nc.gpsimd.memset(q3[:, 1:HP - 1, 0:WP:WP - 1], 0.0)
                nc.vector.tensor_scalar(
                    q3[:, 1:H + 1, 1:W + 1],
                    v[:].rearrange("p (r c) -> p r c", r=H),
                    MAGIC, MAGIC, op0=op.max, op1=op.subtract)
                return q

            qh = quant("h", inv_sh, bh, bf16)
            ql = quant("l", inv_sl, bl, fp8)

            # --- convs: 9 bf16 high taps + (4 DoubleRow + 1) fp8 low taps
            #     per chunk, all accumulating into the chunk's PSUM bank ---
            acc = outs_pool.tile([P, NPIX], f32, tag="acc")
            qh3 = qh[:].rearrange("p (r c) -> p r c", r=HP)
            ql3 = ql[:].rearrange("p (r c) -> p r c", r=HP)
            qlf = ql[:]
            for c in range(NCHUNK):
                r0 = c * ROWS
                seg = (psA[:, c * BANK:c * BANK + NFREE] if c < 4
                       else psB[:, (c - 4) * BANK:(c - 4) * BANK + NFREE])
                for tap in range(NTAPS):
                    kh, kw = divmod(tap, 3)
                    nc.tensor.matmul(
                        seg, qwt_h[:, tap * P:(tap + 1) * P],
                        qh3[:, r0 + kh:r0 + kh + ROWS, kw:kw + W],
                        start=(tap == 0), stop=False)
                for pr in range(4):
                    t0 = 2 * pr
                    kh0, kw0 = divmod(t0, 3)
                    kh1, kw1 = divmod(t0 + 1, 3)
                    o0 = (r0 + kh0) * HP + kw0
                    ds = (r0 + kh1) * HP + kw1 - o0
                    # overlapping strided view [P, 2(pair), 8(rows), 56(cols)]
                    rv = (qlf[:, 0:2 * ROWS * W]
                          .rearrange("p (a b c) -> p a b c", a=2, b=ROWS)
                          .copy())
                    rv.ap = _br.VecI64Pair(
                        [[NPAD, P], [ds, 2], [HP, ROWS], [1, W]])
                    rv.offset = qlf.offset + o0
                    lhsT = (qwt_l[:, t0 * P:(t0 + 2) * P]
                            .rearrange("p (two m) -> p two m", two=2))
                    nc.tensor.matmul(
                        seg, lhsT, rv, start=False, stop=False, perf_mode=DR)
                nc.tensor.matmul(
                    seg, qwt_l[:, 8 * P:9 * P],
                    ql3[:, r0 + 2:r0 + 2 + ROWS, 2:2 + W],
                    start=False, stop=True)
                if c == 3:
                    nc.scalar.mul(
                        acc[:, 0:4 * NFREE].rearrange("p (b k) -> p b k", b=4),
                        psA[:].rearrange("p (b k) -> p b k", b=4)[:, :, 0:NFREE],
                        sv_l[:, 0:1])
                if c == 6:
                    nc.scalar.mul(
                        acc[:, 4 * NFREE:7 * NFREE]
                        .rearrange("p (b k) -> p b k", b=3),
                        psB[:, 0:3 * BANK]
                        .rearrange("p (b k) -> p b k", b=3)[:, :, 0:NFREE],
                        sv_l[:, 0:1])
            nc.sync.dma_start(out=y_d[b], in_=acc[:])


def make_bass(inv_sh, inv_sl, c_svh, c_svl, bpc=BPC):
    import concourse.bacc as bacc
    import concourse.mybir as mybir
    from concourse.tile import TileContext

    f32 = mybir.dt.float32
    nc = bacc.Bacc("TRN2", debug=False)
    x = nc.dram_tensor("x", [bpc, P, NPIX], f32, kind="ExternalInput")
    wh = nc.dram_tensor("w_high", [P, P * NTAPS], f32, kind="ExternalInput")
    wl = nc.dram_tensor("w_low", [P, P * NTAPS], f32, kind="ExternalInput")
    y = nc.dram_tensor("y", [bpc, P, NPIX], f32, kind="ExternalOutput")
    aps = {"x": x.ap(), "w_high": wh.ap(), "w_low": wl.ap(), "y": y.ap()}
    with TileContext(nc) as tc:
        build_program(nc, tc, aps, inv_sh, inv_sl, c_svh, c_svl, bpc=bpc)
    nc.compile()
    return nc


def _scale_consts(act_scale_high, act_scale_low):
    sh = float(np.float32(act_scale_high))
    sl = float(np.float32(act_scale_low))
    inv_sh = float(np.float32(1.0 / np.float64(sh)))
    inv_sl = float(np.float32(1.0 / np.float64(sl)))
    c_svh = float(np.float32(np.float64(sh) / 127.0))
    c_svl = float(np.float32(np.float64(sl) / 7.0))
    return inv_sh, inv_sl, c_svh, c_svl


def _run(x, w_high, w_low, act_scale_high, act_scale_low, trace=False, **kw):
    from concourse import bass_utils

    x = np.ascontiguousarray(np.asarray(x, dtype=np.float32))
    w_high = np.ascontiguousarray(np.asarray(w_high, dtype=np.float32))
    w_low = np.ascontiguousarray(np.asarray(w_low, dtype=np.float32))

    inv_sh, inv_sl, c_svh, c_svl = _scale_consts(act_scale_high, act_scale_low)
    nc = make_bass(inv_sh, inv_sl, c_svh, c_svl)

    wh_flat = w_high.reshape(P, P * NTAPS)
    wl_flat = w_low.reshape(P, P * NTAPS)
    in_maps = []
    for core in range(N_CORES):
        xs = x[core * BPC:(core + 1) * BPC].reshape(BPC, P, NPIX)
        in_maps.append(
            {
                "x": np.ascontiguousarray(xs),
                "w_high": wh_flat,
                "w_low": wl_flat,
            }
        )
    res = bass_utils.run_bass_kernel_spmd(
        nc, in_maps, core_ids=list(range(N_CORES)), trace=trace, **kw
    )
    y = np.concatenate([r["y"].reshape(BPC, P, H, W) for r in res.results], axis=0)
    return y, res


def kernel(x, w_high, w_low, act_scale_high, act_scale_low):
    y, _ = _run(x, w_high, w_low, act_scale_high, act_scale_low)
    return y


# revision 9
# speedup vs baseline: 1.1720x; 1.0055x over previous
"""DRQConv2d (dual-region quantized conv) Trainium2 kernel — v2.

Reference semantics:
  mask  = upsample8(avgpool8(x) >= 0.05)             per (b, c)
  xh    = where(mask, x, 1e-5);  xl = where(mask, 1e-5, x)
  qh    = clip(round(xh/sh), 0, 255) * sh            (uint8 fake-quant)
  ql    = clip(round(xl/sl), 0, 15) * sl             (uint4 fake-quant)
  qwh   = per-oc quant of w_high to +-127,  qwl = per-oc quant of w_low to +-7
  y     = conv3x3(qh, qwh) + conv3x3(ql, qwl)        (pad 1)

Key ideas vs the v1 baseline (151us):
  * Low conv runs in fp8 e4m3 with MatmulPerfMode.DoubleRow: quantized low
    activations (ints 0..15) and weights (ints +-7) are exactly representable
    in e4m3, and DoubleRow packs TWO 3x3 taps (2x128 contraction rows) into
    one PE instruction -> 9 taps in 5 matmuls instead of 9.  The per-oc scale
    ratio sv_h/sv_l is folded into the HIGH (bf16) weights instead of the low
    ones so the low weights stay exact integers; both convs share PSUM banks
    and one final evacuation scale by sv_l (adds ~0.1% error, gate is 2%).
  * The region mask is applied as a CLAMP BOUND instead of a multiply:
    v = x/s + MAGIC (ACT, scale+round fused); u = min(v, MAGIC + qmax*mask)
    (GpSimd, mask expanded only to [P, 7*56] block-row resolution and
    broadcast-viewed); q = max(u, MAGIC) - MAGIC (DVE tensor_scalar, 2x mode).
    Masked-out pixels clamp to exactly MAGIC -> quantize to 0, which matches
    the reference (1e-5 rounds to 0).  This removes the full-res mask
    expansion and the 1x-mode scalar_tensor_tensor of v1.
  * PSUM is laid out as two bank-aligned supertiles (chunks at 512-elem
    stride) so each image needs only 2 strided ACT evacuations + 1 DMA out.

Sharding: data-parallel over batch. 32 images -> 4 per core on 8 cores,
weights replicated; outputs concatenated on host. No collectives.
"""

import numpy as np

P = 128            # channels (both in and out) == partitions
B_TOTAL = 32
N_CORES = 8
BPC = B_TOTAL // N_CORES   # images per core
H = W = 56
HP = WP = H + 2    # zero-padded layout
NPIX = H * W       # 3136
NPAD = HP * WP     # 3364
NTAPS = 9
ROWS = 8
NCHUNK = H // ROWS                    # 7
NFREE = ROWS * W                      # 448 columns per matmul
BANK = 512                            # PSUM bank stride in f32 elements
MAGIC = float(np.float32(1.5 * 2 ** 23))   # fp32 round-to-nearest magic
POOL_K = 8
THRESH = 0.05


def build_program(nc, tc, aps, inv_sh, inv_sl, c_svh, c_svl, bpc=BPC):
    import bass_rust as _br
    import concourse.mybir as mybir
    from concourse.alu_op_type import AluOpType as op
    from concourse.masks import make_identity

    f32 = mybir.dt.float32
    bf16 = mybir.dt.bfloat16
    fp8 = mybir.dt.float8e4
    X = mybir.AxisListType.X
    DR = mybir.MatmulPerfMode.DoubleRow
    IDENT = mybir.ActivationFunctionType.Identity

    x_d, wh_d, wl_d, y_d = aps["x"], aps["w_high"], aps["w_low"], aps["y"]
    sum_thresh = float(np.float32(THRESH) * POOL_K * POOL_K)  # exact pow2 scale

    with (
        tc.tile_pool(name="consts", bufs=1) as consts,
        tc.tile_pool(name="wtmp", bufs=1) as wtmp,
        tc.tile_pool(name="psum", bufs=1, space="PSUM") as psum_pool,
        tc.tile_pool(name="acts", bufs=2) as acts,
        tc.tile_pool(name="qtiles", bufs=3) as qtiles,
        tc.tile_pool(name="outs", bufs=2) as outs_pool,
    ):
        identity = consts.tile([P, P], f32)
        make_identity(nc, identity[:])
        magic_ap = consts.tile([P, 1], f32, tag="magic", name="magic")
        nc.vector.memset(magic_ap[:], MAGIC)

        # PSUM: chunks 0-3 in psA banks 0-3, chunks 4-6 in psB banks 0-2;
        # psB bank 3 (the 8th PSUM bank) doubles as transpose/warmup scratch
        # (only touched before the first conv matmul).
        psA = psum_pool.tile([P, 4 * BANK], f32, tag="psA")
        psB = psum_pool.tile([P, 4 * BANK], f32, tag="psB")
        tp = psB[:, 3 * BANK:4 * BANK]

        # ---------------- weight prep ----------------
        wnat_h = wtmp.tile([P, P * NTAPS], f32, tag="wnat_h")
        wnat_l = wtmp.tile([P, P * NTAPS], f32, tag="wnat_l")
        nc.sync.dma_start(out=wnat_h[:], in_=wh_d)
        nc.sync.dma_start(out=wnat_l[:], in_=wl_d)

        def pp(tag):
            return consts.tile([P, 1], f32, tag=tag, name=tag)

        am_h, am_l = pp("am_h"), pp("am_l")
        nc.vector.tensor_reduce(
            am_h[:], wnat_h[:], axis=X, op=op.max, apply_absolute_value=True)
        nc.vector.tensor_reduce(
            am_l[:], wnat_l[:], axis=X, op=op.max, apply_absolute_value=True)
        sv_l, sv_h = pp("sv_l"), pp("sv_h")
        nc.vector.tensor_scalar_mul(sv_l[:], am_l[:], c_svl)
        nc.vector.tensor_scalar_mul(sv_h[:], am_h[:], c_svh)
        rcp_svl, ratio = pp("rcp_svl"), pp("ratio")
        nc.vector.reciprocal(rcp_svl[:], sv_l[:])
        nc.vector.tensor_tensor(ratio[:], sv_h[:], rcp_svl[:], op=op.mult)
        rcp_h, rs_h = pp("rcp_h"), pp("rs_h")
        nc.vector.reciprocal(rcp_h[:], am_h[:])
        nc.vector.tensor_scalar_mul(rs_h[:], rcp_h[:], 127.0)
        rcp_l, rs_l = pp("rcp_l"), pp("rs_l")
        nc.vector.reciprocal(rcp_l[:], am_l[:])
        nc.vector.tensor_scalar_mul(rs_l[:], rcp_l[:], 7.0)

        # integer-quantize in natural [oc, ic*9] layout, in place
        nc.vector.tensor_scalar(
            wnat_h[:], wnat_h[:], rs_h[:, 0:1], MAGIC, op0=op.mult, op1=op.add)
        nc.vector.tensor_scalar(
            wnat_h[:], wnat_h[:], MAGIC, 127.0, op0=op.subtract, op1=op.min)
        # fold sv_h/sv_l into the high weights so both convs share one
        # output scale (sv_l); low weights stay exact fp8 integers
        nc.vector.tensor_scalar(
            wnat_h[:], wnat_h[:], -127.0, ratio[:, 0:1], op0=op.max, op1=op.mult)
        nc.vector.tensor_scalar(
            wnat_l[:], wnat_l[:], rs_l[:, 0:1], MAGIC, op0=op.mult, op1=op.add)
        nc.vector.tensor_scalar(
            wnat_l[:], wnat_l[:], MAGIC, 7.0, op0=op.subtract, op1=op.min)
        nc.vector.tensor_scalar_max(wnat_l[:], wnat_l[:], -7.0)

        # transpose each tap [oc, ic] -> [ic, oc]; cast to bf16 / fp8e4
        qwt_h = consts.tile([P, NTAPS * P], bf16, tag="qwt_h")
        qwt_l = consts.tile([P, NTAPS * P], fp8, tag="qwt_l")
        for wq, qwt in ((wnat_h, qwt_h), (wnat_l, qwt_l)):
            wv = wq[:].rearrange("p (i t) -> p t i", t=NTAPS)
            for base in range(0, NTAPS, 4):
                n = min(4, NTAPS - base)
                for j in range(n):
                    nc.tensor.transpose(
                        tp[:, j * P:(j + 1) * P], wv[:, base + j, :], identity[:])
                nc.vector.tensor_copy(
                    out=qwt[:, base * P:(base + n) * P], in_=tp[:, :n * P])

        # PE warm-up: HAM un-throttles after ~3.4us of sustained activity
        for i in range(28):
            nc.tensor.matmul(
                tp[:, 0:P], identity[:], identity[:],
                start=(i == 0), stop=(i == 27))

        # ---------------- per-image pipeline ----------------
        # prep(b) runs the mask/quant chains (DVE/ACT/GpSimd); conv(b) runs
        # the matmuls + PSUM evacuation (PE/ACT).  prep(b+1) is EMITTED
        # before conv(b): engines execute their streams in order, so this
        # keeps image b+1's quant passes ahead of image b's evacuations in
        # the ACT/DVE queues (software pipelining across images).
        xts = {}
        qs = {}

        def issue_x(b):
            if b < bpc and b not in xts:
                xts[b] = acts.tile([P, NPIX], f32, tag="xt", name=f"xt{b}")
                nc.sync.dma_start(out=xts[b][:], in_=x_d[b])

        def prep(b):
            issue_x(b + 1)
            xt = xts[b]

            # --- mask -> clamp bounds at block-row resolution ---
            # (w-blocksum written transposed so h-blocksum is contiguous)
            r1 = acts.tile([P, H * NCHUNK], f32, tag="r1")
            nc.vector.reduce_sum(
                r1[:].rearrange("p (w h) -> p h w", w=NCHUNK),
                xt[:].rearrange("p (r c) -> p r c", c=POOL_K), axis=X)
            r2 = acts.tile([P, NCHUNK * NCHUNK], f32, tag="r2")
            nc.vector.reduce_sum(
                r2[:], r1[:].rearrange("p (g c) -> p g c", c=POOL_K), axis=X)
            mt = acts.tile([P, NCHUNK * NCHUNK], f32, tag="mt")
            nc.vector.tensor_scalar(mt[:], r2[:], sum_thresh, None, op0=op.is_ge)
            m = acts.tile([P, NCHUNK * NCHUNK], f32, tag="m")
            nc.vector.tensor_copy(
                out=m[:], in_=mt[:].rearrange("p (w h) -> p h w", w=NCHUNK))
            # bound_h = MAGIC + 255*m ; bound_l = MAGIC + 15*(1-m)
            bh_blk = acts.tile([P, NCHUNK * NCHUNK], f32, tag="bh_blk")
            nc.vector.tensor_scalar(
                bh_blk[:], m[:], 255.0, MAGIC, op0=op.mult, op1=op.add)
            bl_blk = acts.tile([P, NCHUNK * NCHUNK], f32, tag="bl_blk")
            nc.vector.tensor_scalar(
                bl_blk[:], m[:], -15.0, MAGIC + 15.0, op0=op.mult, op1=op.add)
            bh = acts.tile([P, NCHUNK * W], f32, tag="bh")
            nc.vector.tensor_copy(
                out=bh[:].rearrange("p (g c) -> p g c", c=POOL_K),
                in_=bh_blk[:].unsqueeze(2).broadcast_to(
                    (P, NCHUNK * NCHUNK, POOL_K)))
            bl = acts.tile([P, NCHUNK * W], f32, tag="bl")
            nc.vector.tensor_copy(
                out=bl[:].rearrange("p (g c) -> p g c", c=POOL_K),
                in_=bl_blk[:].unsqueeze(2).broadcast_to(
                    (P, NCHUNK * NCHUNK, POOL_K)))

            # --- quantize: ACT scale+round, GpSimd mask-clamp, DVE finish ---
            def quant(conv, inv_s, bnd, qdt):
                v = acts.tile([P, NPIX], f32, tag=f"v{conv}")
                nc.scalar.activation(
                    v[:], xt[:], IDENT, bias=magic_ap[:, 0:1], scale=inv_s)
                v4 = v[:].rearrange("p (hb r c) -> p hb r c", hb=NCHUNK, r=ROWS)
                bnd4 = (bnd[:].rearrange("p (hb c) -> p hb c", hb=NCHUNK)
                        .unsqueeze(2).broadcast_to((P, NCHUNK, ROWS, W)))
                nc.vector.tensor_tensor(v4, v4, bnd4, op=op.min)
                q = qtiles.tile([P, NPAD], qdt, tag=f"q{conv}")
                q3 = q[:].rearrange("p (r c) -> p r c", r=HP)
                nc.gpsimd.memset(q3[:, 0:HP:HP - 1, :], 0.0)
                nc.gpsimd.memset(q3[:, 1:HP - 1, 0:WP:WP - 1], 0.0)
                nc.vector.tensor_scalar(
                    q3[:, 1:H + 1, 1:W + 1],
                    v[:].rearrange("p (r c) -> p r c", r=H),
                    MAGIC, MAGIC, op0=op.max, op1=op.subtract)
                return q

            qs[b] = (quant("h", inv_sh, bh, bf16),
                     quant("l", inv_sl, bl, fp8))

        def seg_of(c):
            return (psA[:, c * BANK:c * BANK + NFREE] if c < 4
                    else psB[:, (c - 4) * BANK:(c - 4) * BANK + NFREE])

        def conv(b):
            # high phase first (needs only qh), then the fp8 low phase --
            # gives prep(b) maximal slack to finish ql while PE runs high.
            qh, ql = qs.pop(b)
            acc = outs_pool.tile([P, NPIX], f32, tag="acc")
            qh3 = qh[:].rearrange("p (r c) -> p r c", r=HP)
            ql3 = ql[:].rearrange("p (r c) -> p r c", r=HP)
            qlf = ql[:]
            for c in range(NCHUNK):
                r0 = c * ROWS
                seg = seg_of(c)
                for tap in range(NTAPS):
                    kh, kw = divmod(tap, 3)
                    nc.tensor.matmul(
                        seg, qwt_h[:, tap * P:(tap + 1) * P],
                        qh3[:, r0 + kh:r0 + kh + ROWS, kw:kw + W],
                        start=(tap == 0), stop=False)
            for c in range(NCHUNK):
                r0 = c * ROWS
                seg = seg_of(c)
                for pr in range(4):
                    t0 = 2 * pr
                    kh0, kw0 = divmod(t0, 3)
                    kh1, kw1 = divmod(t0 + 1, 3)
                    o0 = (r0 + kh0) * HP + kw0
                    ds = (r0 + kh1) * HP + kw1 - o0
                    # overlapping strided view [P, 2(pair), 8(rows), 56(cols)]
                    rv = (qlf[:, 0:2 * ROWS * W]
                          .rearrange("p (a b c) -> p a b c", a=2, b=ROWS)
                          .copy())
                    rv.ap = _br.VecI64Pair(
                        [[NPAD, P], [ds, 2], [HP, ROWS], [1, W]])
                    rv.offset = qlf.offset + o0
                    lhsT = (qwt_l[:, t0 * P:(t0 + 2) * P]
                            .rearrange("p (two m) -> p two m", two=2))
                    nc.tensor.matmul(
                        seg, lhsT, rv, start=False, stop=False, perf_mode=DR)
                nc.tensor.matmul(
                    seg, qwt_l[:, 8 * P:9 * P],
                    ql3[:, r0 + 2:r0 + 2 + ROWS, 2:2 + W],
                    start=False, stop=True)
                if c == 3:
                    nc.scalar.mul(
                        acc[:, 0:4 * NFREE].rearrange("p (b k) -> p b k", b=4),
                        psA[:].rearrange("p (b k) -> p b k", b=4)[:, :, 0:NFREE],
                        sv_l[:, 0:1])
                if c == 6:
                    nc.scalar.mul(
                        acc[:, 4 * NFREE:7 * NFREE]
                        .rearrange("p (b k) -> p b k", b=3),
                        psB[:, 0:3 * BANK]
                        .rearrange("p (b k) -> p b k", b=3)[:, :, 0:NFREE],
                        sv_l[:, 0:1])
            nc.sync.dma_start(out=y_d[b], in_=acc[:])

        issue_x(0)
        prep(0)
        for b in range(bpc):
            if b + 1 < bpc:
                prep(b + 1)
            conv(b)


def make_bass(inv_sh, inv_sl, c_svh, c_svl, bpc=BPC):
    import concourse.bacc as bacc
    import concourse.mybir as mybir
    from concourse.tile import TileContext

    f32 = mybir.dt.float32
    nc = bacc.Bacc("TRN2", debug=False)
    x = nc.dram_tensor("x", [bpc, P, NPIX], f32, kind="ExternalInput")
    wh = nc.dram_tensor("w_high", [P, P * NTAPS], f32, kind="ExternalInput")
    wl = nc.dram_tensor("w_low", [P, P * NTAPS], f32, kind="ExternalInput")
    y = nc.dram_tensor("y", [bpc, P, NPIX], f32, kind="ExternalOutput")
    aps = {"x": x.ap(), "w_high": wh.ap(), "w_low": wl.ap(), "y": y.ap()}
    with TileContext(nc) as tc:
        build_program(nc, tc, aps, inv_sh, inv_sl, c_svh, c_svl, bpc=bpc)
    nc.compile()
    return nc


def _scale_consts(act_scale_high, act_scale_low):
    sh = float(np.float32(act_scale_high))
    sl = float(np.float32(act_scale_low))
    inv_sh = float(np.float32(1.0 / np.float64(sh)))
    inv_sl = float(np.float32(1.0 / np.float64(sl)))
    c_svh = float(np.float32(np.float64(sh) / 127.0))
    c_svl = float(np.float32(np.float64(sl) / 7.0))
    return inv_sh, inv_sl, c_svh, c_svl


def _run(x, w_high, w_low, act_scale_high, act_scale_low, trace=False, **kw):
    from concourse import bass_utils

    x = np.ascontiguousarray(np.asarray(x, dtype=np.float32))
    w_high = np.ascontiguousarray(np.asarray(w_high, dtype=np.float32))
    w_low = np.ascontiguousarray(np.asarray(w_low, dtype=np.float32))

    inv_sh, inv_sl, c_svh, c_svl = _scale_consts(act_scale_high, act_scale_low)
    nc = make_bass(inv_sh, inv_sl, c_svh, c_svl)

    wh_flat = w_high.reshape(P, P * NTAPS)
    wl_flat = w_low.reshape(P, P * NTAPS)
    in_maps = []
    for core in range(N_CORES):
        xs = x[core * BPC:(core + 1) * BPC].reshape(BPC, P, NPIX)
        in_maps.append(
            {
                "x": np.ascontiguousarray(xs),
                "w_high": wh_flat,
                "w_low": wl_flat,
            }
        )
    res = bass_utils.run_bass_kernel_spmd(
        nc, in_maps, core_ids=list(range(N_CORES)), trace=trace, **kw
    )
    y = np.concatenate([r["y"].reshape(BPC, P, H, W) for r in res.results], axis=0)
    return y, res


def kernel(x, w_high, w_low, act_scale_high, act_scale_low):
    y, _ = _run(x, w_high, w_low, act_scale_high, act_scale_low)
    return y
